# revision 4
# baseline (speedup 1.0000x reference)
"""GNN message-passing kernel for Trainium2 (8 NeuronCores).

Computes, for a graph with N=50000 nodes / E=600000 edges / D=128 features:
    h_in  = segment_sum(h[src], dst, num_segments=N)
    h_out = segment_sum(h[dst], src, num_segments=N)

Host: per direction, sort edges by segment id, pack segment-contiguous runs
into groups of <=128 distinct segments and <=1022 edges (8 chunks of 128 edge
slots; the last >=2 slots of every group are pads so the final index of each
gather stream is non-negative).  Every segment's edges live in exactly one
group, so each group yields FINAL output rows.  Groups from both directions
are uniform work units split evenly across 8 cores.

Device, per group: one `dma_gather` (1024 int16 indices biased by -32768
against a table base of h[32768:]) pulls the edge messages into SBUF; a
one-hot (slot-rank vs iota compare on VectorE) and 8 accumulating matmuls on
TensorE reduce them into [128 slots x 128 feat] PSUM; results are written
densely to DRAM and the host scatters rows to the final arrays.
"""

import sys

sys.path.insert(0, '/opt/trn_rl_repo')

import numpy as np

N = 50000
E = 600000
D = 128
P = 128
T = 8             # chunks (of 128 edge slots) per group
CE = T * P        # slot capacity per group (1024)
CAP = CE - 2      # edge capacity per group (pads guarantee trailing idx >= 0)
B = 4             # groups per msgs tile / idx load
N_CORES = 8
BASE = 32768      # gather table base row (int16 bias)
PAD_RANK = 999.0

_KERNELS = {}
last_run_info = None


def _pack_direction(gather_nodes, seg_nodes):
    order = np.argsort(seg_nodes, kind='stable')
    gs = gather_nodes[order].astype(np.int32)
    deg = np.bincount(seg_nodes, minlength=N)
    present = np.flatnonzero(deg)
    pdeg = deg[present]
    cum = np.concatenate([[0], np.cumsum(pdeg)])
    npres = len(present)

    starts = [0]
    i = 0
    while i < npres:
        j_edge = int(np.searchsorted(cum, cum[i] + CAP, side='right')) - 1
        j = min(i + P, j_edge, npres)
        assert j > i, f"segment {present[i]} degree {pdeg[i]} exceeds {CAP}"
        i = j
        starts.append(i)
    starts = np.asarray(starts)
    n_g = len(starts) - 1

    nodes_per_g = starts[1:] - starts[:-1]
    edges_per_g = cum[starts[1:]] - cum[starts[:-1]]
    g_of_node = np.repeat(np.arange(n_g), nodes_per_g)
    slot_of_node = np.arange(npres) - np.repeat(starts[:-1], nodes_per_g)
    g_of_edge = np.repeat(np.arange(n_g), edges_per_g)
    pos_of_edge = np.arange(len(gs)) - np.repeat(cum[starts[:-1]], edges_per_g)
    slot_of_edge = np.repeat(slot_of_node, pdeg)

    gidx = np.full((n_g, CE), BASE, dtype=np.int32)   # pad -> row BASE (idx16=0)
    rank = np.full((n_g, CE), PAD_RANK, dtype=np.float32)
    flat = g_of_edge * CE + pos_of_edge
    gidx.flat[flat] = gs
    rank.flat[flat] = slot_of_edge.astype(np.float32)
    scat = np.full((n_g, P), -1, dtype=np.int32)
    scat.flat[g_of_node * P + slot_of_node] = present
    return gidx, rank, scat


def _build_device_kernel(G):
    from concourse import bacc, mybir, tile

    n_b = (G + B - 1) // B
    nc = bacc.Bacc("TRN2", target_bir_lowering=False, debug=False,
                   num_devices=N_CORES)
    f32, i32, i16 = mybir.dt.float32, mybir.dt.int32, mybir.dt.int16

    h_d = nc.dram_tensor("h", [N, D], f32, kind="ExternalInput").ap()
    idx_d = nc.dram_tensor("idx16", [P, G * 64], i16, kind="ExternalInput").ap()
    rank_d = nc.dram_tensor("rank", [P, G * T], f32, kind="ExternalInput").ap()
    dense_d = nc.dram_tensor("dense", [P, G, D], f32, kind="ExternalOutput").ap()

    h_base = h_d[BASE:, :]

    with tile.TileContext(nc) as tc:
        with tc.tile_pool(name="const", bufs=1) as const_pool, \
             tc.tile_pool(name="idx", bufs=3) as idx_pool, \
             tc.tile_pool(name="msgs", bufs=2) as msgs_pool, \
             tc.tile_pool(name="oh", bufs=4) as oh_pool, \
             tc.tile_pool(name="outsb", bufs=2) as outsb_pool, \
             tc.tile_pool(name="psum", bufs=8, space="PSUM") as psum_pool:

            sb_rank = const_pool.tile([P, G * T], f32)
            nc.sync.dma_start(out=sb_rank[:], in_=rank_d[:])
            iota_i = const_pool.tile([P, P], i32)
            nc.gpsimd.iota(iota_i[:], pattern=[[1, P]], base=0,
                           channel_multiplier=0)
            iota_f = const_pool.tile([P, P], f32)
            nc.vector.tensor_copy(out=iota_f[:], in_=iota_i[:])

            for b in range(n_b):
                g0 = b * B
                g1 = min(g0 + B, G)
                nb = g1 - g0

                idx_t = idx_pool.tile([P, B * 64], i16, tag="idx")
                nc.sync.dma_start(out=idx_t[:, :nb * 64],
                                  in_=idx_d[:, g0 * 64:g1 * 64])

                msgs = msgs_pool.tile([P, B * T, D], f32, tag="msgs")
                outsb = outsb_pool.tile([P, B * P], f32, tag="outsb")
                for gg in range(nb):
                    g = g0 + gg
                    nc.gpsimd.dma_gather(
                        out_ap=msgs[:, gg * T:(gg + 1) * T, :],
                        in_ap=h_base,
                        idxs_ap=idx_t[:, gg * 64:(gg + 1) * 64],
                        num_idxs=CE,
                        num_idxs_reg=CE,
                        elem_size=D,

                    )
                    oh = oh_pool.tile([P, CE], f32, tag="oh")
                    nc.vector.tensor_tensor(
                        out=oh[:].rearrange("p (t j) -> p t j", j=P),
                        in0=iota_f[:, None, :].to_broadcast([P, T, P]),
                        in1=sb_rank[:, g * T:(g + 1) * T, None].to_broadcast(
                            [P, T, P]),
                        op=mybir.AluOpType.is_equal,
                    )
                    ps = psum_pool.tile([P, P], f32, space="PSUM", tag="ps")
                    for t in range(T):
                        nc.tensor.matmul(
                            out=ps[:],
                            lhsT=oh[:, t * P:(t + 1) * P],
                            rhs=msgs[:, gg * T + t, :],
                            start=(t == 0),
                            stop=(t == T - 1),
                        )
                    nc.scalar.copy(out=outsb[:, gg * P:(gg + 1) * P], in_=ps[:])

                nc.sync.dma_start(
                    out=dense_d[:, g0:g1, :],
                    in_=outsb[:, :nb * P].rearrange("s (g d) -> s g d", d=D),
                )

    nc.compile()
    return nc


def kernel(h, src, dst):
    global last_run_info
    from concourse.bass_utils import run_bass_kernel_spmd

    h32 = np.ascontiguousarray(np.asarray(h, dtype=np.float32))
    src64 = np.asarray(src).astype(np.int64)
    dst64 = np.asarray(dst).astype(np.int64)
    assert h32.shape == (N, D) and src64.shape == (E,) and dst64.shape == (E,)

    g0, r0, s0 = _pack_direction(src64, dst64)
    g1, r1, s1 = _pack_direction(dst64, src64)
    s1 = np.where(s1 >= 0, s1 + N, -1)
    gidx = np.concatenate([g0, g1])
    rank = np.concatenate([r0, r1])
    scat = np.concatenate([s0, s1])

    n_g_tot = gidx.shape[0]
    n_g_pad = -(-n_g_tot // N_CORES) * N_CORES
    if n_g_pad > n_g_tot:
        pad = n_g_pad - n_g_tot
        gidx = np.concatenate([gidx, np.full((pad, CE), BASE, np.int32)])
        rank = np.concatenate([rank, np.full((pad, CE), PAD_RANK, np.float32)])
        scat = np.concatenate([scat, np.full((pad, P), -1, np.int32)])
    G = n_g_pad // N_CORES

    in_maps = []
    scat_cores = []
    for c in range(N_CORES):
        sl = slice(c * G, (c + 1) * G)
        gidx_c = gidx[sl].reshape(-1)
        idx16 = (gidx_c - BASE).astype(np.int16)
        w = np.ascontiguousarray(idx16.reshape(-1, 16).T)      # [16, G*64]
        w128 = np.ascontiguousarray(np.tile(w, (8, 1)))        # [128, G*64]
        rank_c = np.ascontiguousarray(
            rank[sl].reshape(-1, P).T.astype(np.float32))      # [128, G*T]
        in_maps.append({"h": h32, "idx16": w128, "rank": rank_c})
        scat_cores.append(scat[sl])

    if G not in _KERNELS:
        _KERNELS[G] = _build_device_kernel(G)
    nc = _KERNELS[G]

    res = run_bass_kernel_spmd(nc, in_maps, list(range(N_CORES)))
    last_run_info = res

    h_all = np.zeros((2 * N, D), dtype=np.float32)
    for c in range(N_CORES):
        dense = res.results[c]["dense"]          # [P(slot), G, D]
        sc = scat_cores[c]                       # [G, P]
        mask = sc >= 0
        h_all[sc[mask]] = dense.transpose(1, 0, 2)[mask]
    return h_all[:N], h_all[N:]
